# revision 7
# baseline (speedup 1.0000x reference)
"""Trainium2 Bass kernel for: out[b,o] = sum_f x[b,f]*weight[o,f]*m[b,o,f] + bias[o].

Strategy (pure data parallel over batch, 8 cores, 32 batch rows each):
  - Stream m as 256 tiles of [128(o), 1024(f)] per core (contiguous 512KB DMAs).
  - DVE: wm = m_tile * weight_tile (weight resident in SBUF).
  - PE: transpose each 128x128 block of wm into PSUM ([f, o] layout).
  - ACT: copy transposed blocks PSUM->SBUF.
  - PE: out_row[1,128] = sum_j xT_col_j^T @ wmT_j  (x folded into the matmul).
  - DVE: out_row += bias while moving PSUM->SBUF; DMA to DRAM.
"""

import numpy as np

BATCH, FOUT, FIN = 256, 1024, 1024
NCORES = 8
B_LOC = BATCH // NCORES  # 32
P = 128
NOT = FOUT // P  # 8 o-tiles per batch row
NJ = FIN // P    # 8 f-blocks

_NC_CACHE = {}


def _build(b_loc=B_LOC):
    import concourse.bass as bass
    import concourse.bacc as bacc
    import concourse.mybir as mybir
    from concourse.tile import TileContext
    from concourse.masks import make_identity

    nc = bacc.Bacc("TRN2")
    m_d = nc.dram_tensor("m_in", [b_loc, FOUT, FIN], mybir.dt.float32,
                         kind="ExternalInput")
    wg_d = nc.dram_tensor("wg_in", [P, NOT * FIN], mybir.dt.float32,
                          kind="ExternalInput")
    xTg_d = nc.dram_tensor("xTg_in", [P, NJ * b_loc], mybir.dt.float32,
                           kind="ExternalInput")
    b_d = nc.dram_tensor("b_in", [1, FOUT], mybir.dt.float32,
                         kind="ExternalInput")
    out_d = nc.dram_tensor("out", [b_loc, FOUT], mybir.dt.float32,
                           kind="ExternalOutput")

    with TileContext(nc) as tc:
        with (
            tc.tile_pool(name="const", bufs=1) as constp,
            tc.tile_pool(name="mp", bufs=4) as mp,
            tc.tile_pool(name="wmp", bufs=4) as wmp,
            tc.tile_pool(name="wmtp", bufs=4) as wmtp,
            tc.tile_pool(name="orow", bufs=4) as orowp,
            tc.tile_pool(name="pst", bufs=4, space="PSUM") as pst,
            tc.tile_pool(name="pso", bufs=4, space="PSUM") as pso,
        ):
            ident = constp.tile([P, P], mybir.dt.float32, tag="ident")
            make_identity(nc, ident)
            wg_sb = constp.tile([P, NOT * FIN], mybir.dt.float32, tag="wg")
            nc.gpsimd.dma_start(wg_sb, wg_d[:, :])
            xTg_sb = constp.tile([P, NJ * b_loc], mybir.dt.float32, tag="xTg")
            nc.gpsimd.dma_start(xTg_sb, xTg_d[:, :])
            bias_sb = constp.tile([1, FOUT], mybir.dt.float32, tag="bias")
            nc.gpsimd.dma_start(bias_sb, b_d[:, :])

            for b in range(b_loc):
                for ot in range(NOT):
                    mt = mp.tile([P, FIN], mybir.dt.float32, tag="mt")
                    nc.sync.dma_start(mt, m_d[b, ot * P:(ot + 1) * P, :])
                    wm = wmp.tile([P, FIN], mybir.dt.float32, tag="wm")
                    nc.vector.tensor_tensor(
                        wm, mt, wg_sb[:, ot * FIN:(ot + 1) * FIN],
                        mybir.AluOpType.mult)
                    wmT = wmtp.tile([P, FIN], mybir.dt.float32, tag="wmT")
                    for g in range(2):
                        ps = pst.tile([P, 512], mybir.dt.float32, tag="pst")
                        for jj in range(4):
                            j = g * 4 + jj
                            nc.tensor.transpose(
                                ps[:, jj * P:(jj + 1) * P],
                                wm[:, j * P:(j + 1) * P], ident)
                        nc.scalar.copy(wmT[:, g * 512:(g + 1) * 512], ps)
                    po = pso.tile([1, P], mybir.dt.float32, tag="po")
                    for j in range(NJ):
                        col = j * b_loc + b
                        nc.tensor.matmul(po, xTg_sb[:, col:col + 1],
                                         wmT[:, j * P:(j + 1) * P],
                                         start=(j == 0), stop=(j == NJ - 1))
                    orow = orowp.tile([1, P], mybir.dt.float32, tag="orow")
                    nc.vector.tensor_tensor(
                        orow, po, bias_sb[:, ot * P:(ot + 1) * P],
                        mybir.AluOpType.add)
                    nc.sync.dma_start(out_d[b:b + 1, ot * P:(ot + 1) * P],
                                      orow)
    nc.finalize()
    return nc


def _get_nc(b_loc=B_LOC):
    if b_loc not in _NC_CACHE:
        _NC_CACHE[b_loc] = _build(b_loc)
    return _NC_CACHE[b_loc]


def _prep_core_inputs(x_c, m_c, weight, bias, b_loc):
    wg = np.ascontiguousarray(
        weight.reshape(NOT, P, FIN).transpose(1, 0, 2).reshape(P, NOT * FIN))
    xTg = np.ascontiguousarray(
        x_c.T.reshape(NJ, P, b_loc).transpose(1, 0, 2).reshape(P, NJ * b_loc))
    return {
        "m_in": np.ascontiguousarray(m_c),
        "wg_in": wg,
        "xTg_in": xTg,
        "b_in": np.ascontiguousarray(bias.reshape(1, FOUT)),
    }


def kernel(x, m, weight, bias, _trace=False, _trace_kwargs=None):
    from concourse import bass_utils
    nc = _get_nc()
    x = np.asarray(x, np.float32)
    m = np.asarray(m, np.float32)
    weight = np.asarray(weight, np.float32)
    bias = np.asarray(bias, np.float32)
    in_maps = []
    for c in range(NCORES):
        bs = slice(c * B_LOC, (c + 1) * B_LOC)
        in_maps.append(_prep_core_inputs(x[bs], m[bs], weight, bias, B_LOC))
    res = bass_utils.run_bass_kernel_spmd(
        nc, in_maps, core_ids=list(range(NCORES)),
        trace=_trace, **(_trace_kwargs or {}))
    out = np.concatenate([r["out"] for r in res.results], axis=0)
    if _trace:
        return out, res
    return out
